# revision 4
# baseline (speedup 1.0000x reference)
"""Single-head causal attention (B=4, T=2048, C=1024, fp32) on 8 Trainium2 cores.

v4 = v3 + pairwise K/V exchange: the two cores of a batch each compute K and
V for only their own half of the sequence (k rows [1024h, 1024h+1024)), then
swap halves with an AllGather over replica pairs [[0,1],[2,3],[4,5],[6,7]]
through DRAM bounce buffers. This halves the K and V projection matmul work
per core (the baseline computed full K,V on both cores of a pair).

Phase order V-half -> K-half -> Q so each exchange is in flight while the
next projection phase computes; gathered halves are read back rank-major,
which equals k-major, so attention addressing is core-uniform (SPMD).

Everything else (bf16 compute path, 8 slots of 128-row q-blocks with 16-2s
k-units, masks on last 2 units, psum-direct exp, no running max) is as v3.
"""

import os
import sys

import numpy as np

for _p in ("/opt/trn_rl_repo", os.path.expanduser("~/.axon_site/_ro/trn_rl_repo")):
    if os.path.isdir(_p) and _p not in sys.path:
        sys.path.insert(0, _p)

B, T, C = 4, 2048, 1024
NSLOT = 8
SLOT_UNITS = [16 - 2 * s for s in range(NSLOT)]      # [16,14,12,10,8,6,4,2]
ASSIGN = {
    0: [(15 - 2 * s) if s % 2 == 0 else (14 - 2 * s) for s in range(NSLOT)],
    1: [(14 - 2 * s) if s % 2 == 0 else (15 - 2 * s) for s in range(NSLOT)],
}
SCALE = float(C) ** -0.5
NMASK = 2 * NSLOT
PAIRS = [[0, 1], [2, 3], [4, 5], [6, 7]]

_CACHE = {}


def _build_nc(reps=1):
    import concourse.tile as tile
    from concourse import bacc, mybir
    from contextlib import ExitStack

    f32 = mybir.dt.float32
    bf16 = mybir.dt.bfloat16
    Exp = mybir.ActivationFunctionType.Exp
    Copy = mybir.ActivationFunctionType.Copy

    nc = bacc.Bacc("TRN2", target_bir_lowering=False, debug=False)

    # xhT: this core's own k-half of x^T; xqT: this core's q-blocks of x^T
    xhT = nc.dram_tensor("xhT", [C, 1024], bf16, kind="ExternalInput").ap()
    xqT = nc.dram_tensor("xqT", [C, 1024], bf16, kind="ExternalInput").ap()
    wkT = nc.dram_tensor("wkT", [C, C], bf16, kind="ExternalInput").ap()
    wqT = nc.dram_tensor("wqT", [C, C], bf16, kind="ExternalInput").ap()
    wvT = nc.dram_tensor("wvT", [C, C], bf16, kind="ExternalInput").ap()
    masks = nc.dram_tensor("masks", [NMASK, 128, 128], bf16, kind="ExternalInput").ap()
    out = nc.dram_tensor("out", [1024, C], f32, kind="ExternalOutput").ap()

    def load_half(pool, tag, name, dram_ap, cols, bufs=None):
        t = pool.tile([128, 4 * cols], bf16, tag=tag, name=name, bufs=bufs)
        nc.sync.dma_start(
            out=t[:].rearrange("p (a m) -> p a m", a=4),
            in_=dram_ap.rearrange("(a p) m -> p a m", p=128),
        )
        return t

    def load_w(pool, name, dram_ap):
        return (load_half(pool, "wA", name + "A", dram_ap[0:512, :], C),
                load_half(pool, "wB", name + "B", dram_ap[512:1024, :], C))

    def load_xfull(pool, tag, dram_ap):
        """[1024, 1024] DRAM -> one [128, 8*1024] bf16 tile, two half DMAs."""
        t = pool.tile([128, 8 * 1024], bf16, tag=tag, name=tag)
        for hh in range(2):
            nc.sync.dma_start(
                out=t[:, 4096 * hh:4096 * (hh + 1)].rearrange(
                    "p (a m) -> p a m", a=4),
                in_=dram_ap[512 * hh:512 * (hh + 1), :].rearrange(
                    "(a p) m -> p a m", p=128),
            )
        return t

    with tile.TileContext(nc) as tc:
      for rep in range(reps):
        with ExitStack() as ctx:
            # ---- persistent SBUF arrays ---------------------------------
            kt_pool = ctx.enter_context(tc.tile_pool(name="ktp", bufs=1))
            v_pool = ctx.enter_context(tc.tile_pool(name="vp", bufs=1))
            qt_pool = ctx.enter_context(tc.tile_pool(name="qtp", bufs=1))
            misc_pool = ctx.enter_context(tc.tile_pool(name="miscp", bufs=1))

            KT = [kt_pool.tile([128, T], bf16, tag=f"kt{i}", name=f"kt{i}")
                  for i in range(8)]
            V = [v_pool.tile([128, C], bf16, tag=f"v{i}", name=f"v{i}")
                 for i in range(16)]
            QT = [qt_pool.tile([128, 1024], bf16, tag=f"qt{i}", name=f"qt{i}")
                  for i in range(8)]

            msk = misc_pool.tile([128, NMASK * 128], bf16, name="msk")
            nc.sync.dma_start(
                out=msk[:].rearrange("p (u m) -> p u m", u=NMASK),
                in_=masks[:, :, :].rearrange("u p m -> p u m"),
            )
            ones = misc_pool.tile([128, 2], bf16, name="ones")
            nc.vector.memset(ones[:], 1.0)

            def wslice(wh, ci, lo, hi):
                return wh[ci // 4][:, C * (ci % 4) + lo: C * (ci % 4) + hi]

            with tc.tile_pool(name="wp", bufs=2) as w_pool, \
                 tc.tile_pool(name="xp", bufs=1) as x_pool, \
                 tc.tile_pool(name="locp", bufs=1) as loc_pool, \
                 tc.tile_pool(name="dram", bufs=1, space="DRAM") as dram, \
                 tc.psum_pool(name="pproj", bufs=4) as pp:

                wv = load_w(w_pool, "wv", wvT)
                xh = load_xfull(x_pool, "xh", xhT)
                Vloc = [loc_pool.tile([128, C], bf16, tag=f"vl{i}", name=f"vl{i}")
                        for i in range(8)]
                KTloc = [loc_pool.tile([128, 1024], bf16, tag=f"kl{i}",
                                       name=f"kl{i}") for i in range(8)]

                # ---- phase V-half: Vloc[kcl] = xh[:,kcl].T @ wv ----------
                for w in range(4):
                    for kc2 in range(2):
                        kcl = 2 * w + kc2
                        for half in range(2):
                            ps = pp.tile([128, 512], f32, tag="pv",
                                         name=f"vps{kcl}_{half}")
                            for ci in range(8):
                                nc.tensor.matmul(
                                    ps[:],
                                    xh[:, 1024 * ci + 256 * w + 128 * kc2:
                                       1024 * ci + 256 * w + 128 * (kc2 + 1)],
                                    wslice(wv, ci, 512 * half, 512 * (half + 1)),
                                    start=(ci == 0), stop=(ci == 7),
                                )
                            nc.vector.tensor_copy(
                                Vloc[kcl][:, 512 * half:512 * (half + 1)], ps[:])

                # stage + exchange V halves
                vst_in = dram.tile([128, 8 * 1024], bf16, name="vst_in")
                vst_out = dram.tile([256, 8 * 1024], bf16, name="vst_out")
                for kcl in range(8):
                    nc.sync.dma_start(
                        out=vst_in[:, 1024 * kcl:1024 * (kcl + 1)],
                        in_=Vloc[kcl][:])
                nc.gpsimd.collective_compute(
                    "AllGather", mybir.AluOpType.bypass, replica_groups=PAIRS,
                    ins=[vst_in.opt()], outs=[vst_out.opt()])
                for r in range(2):
                    for kcl in range(8):
                        nc.sync.dma_start(
                            out=V[8 * r + kcl][:],
                            in_=vst_out[128 * r:128 * (r + 1),
                                        1024 * kcl:1024 * (kcl + 1)])

                # ---- phase K-half: KTloc[co] = wk.T @ xh -----------------
                wk = load_w(w_pool, "wk", wkT)
                for w in range(2):
                    for co in range(8):
                        ps = pp.tile([128, 512], f32, tag="pk", name=f"kps{w}_{co}")
                        for ci in range(8):
                            nc.tensor.matmul(
                                ps[:],
                                wslice(wk, ci, 128 * co, 128 * (co + 1)),
                                xh[:, 1024 * ci + 512 * w:1024 * ci + 512 * (w + 1)],
                                start=(ci == 0), stop=(ci == 7),
                            )
                        nc.scalar.copy(KTloc[co][:, 512 * w:512 * (w + 1)], ps[:])

                # stage + exchange K halves
                kst_in = dram.tile([128, 8 * 1024], bf16, name="kst_in")
                kst_out = dram.tile([256, 8 * 1024], bf16, name="kst_out")
                for co in range(8):
                    nc.sync.dma_start(
                        out=kst_in[:, 1024 * co:1024 * (co + 1)],
                        in_=KTloc[co][:])
                nc.gpsimd.collective_compute(
                    "AllGather", mybir.AluOpType.bypass, replica_groups=PAIRS,
                    ins=[kst_in.opt()], outs=[kst_out.opt()])
                for r in range(2):
                    for co in range(8):
                        nc.sync.dma_start(
                            out=KT[co][:, 1024 * r:1024 * (r + 1)],
                            in_=kst_out[128 * r:128 * (r + 1),
                                        1024 * co:1024 * (co + 1)])

                # ---- phase Q: QT[co] = wq.T @ xq -------------------------
                wq = load_w(w_pool, "wq", wqT)
                xq = load_xfull(x_pool, "xq", xqT)
                for i in range(2):
                    for co in range(8):
                        ps = pp.tile([128, 512], f32, tag="pk", name=f"qps{i}_{co}")
                        for ci in range(8):
                            nc.tensor.matmul(
                                ps[:],
                                wslice(wq, ci, 128 * co, 128 * (co + 1)),
                                xq[:, 1024 * ci + 512 * i:1024 * ci + 512 * (i + 1)],
                                start=(ci == 0), stop=(ci == 7),
                            )
                        nc.scalar.copy(QT[co][:, 512 * i:512 * (i + 1)], ps[:])

            # ---- attention ----------------------------------------------
            with tc.tile_pool(name="ptp", bufs=3) as pt_pool, \
                 tc.tile_pool(name="outp", bufs=2) as out_pool, \
                 tc.tile_pool(name="linvp", bufs=2) as linv_pool, \
                 tc.psum_pool(name="sp", bufs=3) as sp, \
                 tc.psum_pool(name="op", bufs=2) as op, \
                 tc.psum_pool(name="lp", bufs=1) as lp:

                def do_S_pair(s, t, n):
                    # S chains for units (2t, 2t+1) into one [128,256] psum;
                    # a single exp (and mask-add for the last pair) covers both
                    s_ps = sp.tile([128, 256], f32, tag="s", name=f"s{s}_{t}")
                    for u in range(2):
                        j = 2 * t + u
                        for ci in range(8):
                            nc.tensor.matmul(
                                s_ps[:, 128 * u:128 * (u + 1)],
                                KT[ci][:, 128 * j:128 * (j + 1)],
                                QT[ci][:, 128 * s:128 * (s + 1)],
                                start=(ci == 0), stop=(ci == 7),
                            )
                    if t == n // 2 - 1:
                        sm = pt_pool.tile([128, 256], f32, tag="sm",
                                          name=f"sm{s}_{t}")
                        nc.vector.tensor_add(sm[:], s_ps[:],
                                             msk[:, 256 * s:256 * (s + 1)])
                        src = sm
                    else:
                        src = s_ps
                    pm = pt_pool.tile([128, 256], bf16, tag="pm", name=f"pm{s}_{t}")
                    nc.scalar.activation(pm[:], src[:], Exp, scale=SCALE)
                    return pm

                for s in range(NSLOT):
                    n = SLOT_UNITS[s]
                    o_ps = op.tile([128, C], f32, tag="o", name=f"o{s}")
                    l_ps = lp.tile([128, 2], f32, tag="l", name=f"l{s}")

                    for t in range(n // 2):
                        pm = do_S_pair(s, t, n)
                        for u in range(2):
                            j = 2 * t + u
                            pmu = pm[:, 128 * u:128 * (u + 1)]
                            first, last = (j == 0), (j == n - 1)
                            nc.tensor.matmul(o_ps[:, 0:512], pmu,
                                             V[j][:, 0:512],
                                             start=first, stop=last)
                            nc.tensor.matmul(o_ps[:, 512:1024], pmu,
                                             V[j][:, 512:1024],
                                             start=first, stop=last)
                            nc.tensor.matmul(l_ps[:], pmu, ones[:],
                                             start=first, stop=last)

                    linv = linv_pool.tile([128, 1], f32, tag="linv",
                                          name=f"linv{s}")
                    nc.vector.reciprocal(linv[:], l_ps[:, 0:1])
                    for half in range(2):
                        o_sb = out_pool.tile([128, 512], f32, tag=f"ost{half}",
                                             name=f"ost{s}_{half}")
                        nc.scalar.activation(o_sb[:],
                                             o_ps[:, 512 * half:512 * (half + 1)],
                                             Copy, scale=linv[:])
                        nc.sync.dma_start(
                            out=out[128 * s:128 * (s + 1),
                                    512 * half:512 * (half + 1)],
                            in_=o_sb[:],
                        )
    nc.finalize()
    return nc


def _masks_for_half(h):
    import ml_dtypes
    m = np.zeros((NMASK, 128, 128), np.float32)
    for s in range(NSLOT):
        n = SLOT_UNITS[s]
        g = ASSIGN[h][s]
        for d in range(2):
            j = n - 2 + d
            ks = 128 * j + np.arange(128)[:, None]
            qs = 128 * g + np.arange(128)[None, :]
            m[2 * s + d] = np.where(ks <= qs, 0.0, -30000.0)
    return m.astype(ml_dtypes.bfloat16)


def _get_built():
    if "nc" not in _CACHE:
        _CACHE["nc"] = _build_nc()
        _CACHE["masks"] = {h: _masks_for_half(h) for h in (0, 1)}
    return _CACHE["nc"], _CACHE["masks"]


def make_in_maps(x, Wk, Wq, Wv, mks):
    import ml_dtypes
    bf = ml_dtypes.bfloat16
    x = np.asarray(x, np.float32)
    wkT = np.ascontiguousarray(np.asarray(Wk, np.float32).T.astype(bf))
    wqT = np.ascontiguousarray(np.asarray(Wq, np.float32).T.astype(bf))
    wvT = np.ascontiguousarray(np.asarray(Wv, np.float32).T.astype(bf))

    in_maps = []
    for core in range(8):
        b, h = core // 2, core % 2
        xT_b = np.ascontiguousarray(x[b].T.astype(bf))
        gs = ASSIGN[h]
        xqT = np.ascontiguousarray(
            np.concatenate([xT_b[:, 128 * g:128 * (g + 1)] for g in gs], axis=1)
        )
        xhT = np.ascontiguousarray(xT_b[:, 1024 * h:1024 * (h + 1)])
        in_maps.append({
            "xhT": xhT, "xqT": xqT,
            "wkT": wkT, "wqT": wqT, "wvT": wvT,
            "masks": mks[h],
        })
    return in_maps


def kernel(x, Wk, Wq, Wv, **_ignored):
    from concourse.bass_utils import run_bass_kernel_spmd

    nc, mks = _get_built()
    in_maps = make_in_maps(x, Wk, Wq, Wv, mks)
    res = run_bass_kernel_spmd(nc, in_maps, core_ids=list(range(8)))
    _CACHE["last_res"] = res

    out = np.empty((B, T, C), np.float32)
    for core in range(8):
        b, h = core // 2, core % 2
        o = res.results[core]["out"]
        for s, g in enumerate(ASSIGN[h]):
            out[b, 128 * g:128 * (g + 1), :] = o[128 * s:128 * (s + 1), :]
    return out


# revision 6
# speedup vs baseline: 1.3388x; 1.3388x over previous
"""Single-head causal attention (B=4, T=2048, C=1024, fp32) on 8 Trainium2 cores.

v4 = v3 + pairwise K/V exchange: the two cores of a batch each compute K and
V for only their own half of the sequence (k rows [1024h, 1024h+1024)), then
swap halves with an AllGather over replica pairs [[0,1],[2,3],[4,5],[6,7]]
through DRAM bounce buffers. This halves the K and V projection matmul work
per core (the baseline computed full K,V on both cores of a pair).

Phase order V-half -> K-half -> Q so each exchange is in flight while the
next projection phase computes; gathered halves are read back rank-major,
which equals k-major, so attention addressing is core-uniform (SPMD).

Everything else (bf16 compute path, 8 slots of 128-row q-blocks with 16-2s
k-units, masks on last 2 units, psum-direct exp, no running max) is as v3.
"""

import os
import sys

import numpy as np

for _p in ("/opt/trn_rl_repo", os.path.expanduser("~/.axon_site/_ro/trn_rl_repo")):
    if os.path.isdir(_p) and _p not in sys.path:
        sys.path.insert(0, _p)

B, T, C = 4, 2048, 1024
NSLOT = 8
SLOT_UNITS = [16 - 2 * s for s in range(NSLOT)]      # [16,14,12,10,8,6,4,2]
ASSIGN = {
    0: [(15 - 2 * s) if s % 2 == 0 else (14 - 2 * s) for s in range(NSLOT)],
    1: [(14 - 2 * s) if s % 2 == 0 else (15 - 2 * s) for s in range(NSLOT)],
}
SCALE = float(C) ** -0.5
NMASK = 2 * NSLOT
PAIRS = [[0, 1], [2, 3], [4, 5], [6, 7]]

_CACHE = {}


def _build_nc(reps=1):
    import concourse.tile as tile
    from concourse import bacc, mybir
    from contextlib import ExitStack

    f32 = mybir.dt.float32
    bf16 = mybir.dt.bfloat16
    Exp = mybir.ActivationFunctionType.Exp
    Copy = mybir.ActivationFunctionType.Copy

    nc = bacc.Bacc("TRN2", target_bir_lowering=False, debug=False)

    # xhT: this core's own k-half of x^T; xqT: this core's q-blocks of x^T
    xT = nc.dram_tensor("xT", [C, T], bf16, kind="ExternalInput").ap()
    xN = nc.dram_tensor("xN", [T, C], bf16, kind="ExternalInput").ap()
    xqT = nc.dram_tensor("xqT", [C, 1024], bf16, kind="ExternalInput").ap()
    # "wqT" carries M = Wq^T @ Wk (host-folded): S = (x_q M) x^T
    wqT = nc.dram_tensor("wqT", [C, C], bf16, kind="ExternalInput").ap()
    wvT = nc.dram_tensor("wvT", [C, C], bf16, kind="ExternalInput").ap()
    masks = nc.dram_tensor("masks", [NMASK, 128, 128], bf16, kind="ExternalInput").ap()
    out = nc.dram_tensor("out", [1024, C], f32, kind="ExternalOutput").ap()

    def load_half(pool, tag, name, dram_ap, cols, bufs=None):
        t = pool.tile([128, 4 * cols], bf16, tag=tag, name=name, bufs=bufs)
        nc.sync.dma_start(
            out=t[:].rearrange("p (a m) -> p a m", a=4),
            in_=dram_ap.rearrange("(a p) m -> p a m", p=128),
        )
        return t

    def load_w(pool, name, dram_ap):
        return (load_half(pool, "wA", name + "A", dram_ap[0:512, :], C),
                load_half(pool, "wB", name + "B", dram_ap[512:1024, :], C))

    def load_xfull(pool, tag, dram_ap):
        """[1024, 1024] DRAM -> one [128, 8*1024] bf16 tile, two half DMAs."""
        t = pool.tile([128, 8 * 1024], bf16, tag=tag, name=tag)
        for hh in range(2):
            nc.sync.dma_start(
                out=t[:, 4096 * hh:4096 * (hh + 1)].rearrange(
                    "p (a m) -> p a m", a=4),
                in_=dram_ap[512 * hh:512 * (hh + 1), :].rearrange(
                    "(a p) m -> p a m", p=128),
            )
        return t

    with tile.TileContext(nc) as tc:
      for rep in range(reps):
        with ExitStack() as ctx:
            # ---- persistent SBUF arrays ---------------------------------
            kt_pool = ctx.enter_context(tc.tile_pool(name="ktp", bufs=1))
            v_pool = ctx.enter_context(tc.tile_pool(name="vp", bufs=1))
            qt_pool = ctx.enter_context(tc.tile_pool(name="qtp", bufs=1))
            misc_pool = ctx.enter_context(tc.tile_pool(name="miscp", bufs=1))

            xt_all = kt_pool.tile([128, 8 * T], bf16, tag="xta", name="xta")
            for hh in range(2):
                nc.sync.dma_start(
                    out=xt_all[:].rearrange("p (a m) -> p a m", a=8)[
                        :, 4 * hh:4 * (hh + 1), :],
                    in_=xT[512 * hh:512 * (hh + 1), :].rearrange(
                        "(a p) m -> p a m", p=128))
            xk_all = v_pool.tile([128, 16 * C], bf16, tag="xka", name="xka")
            nc.sync.dma_start(
                out=xk_all[:].rearrange("p (a m) -> p a m", a=16),
                in_=xN.rearrange("(a p) m -> p a m", p=128))
            ut_all = v_pool.tile([128, 8 * 1024], bf16, tag="uta", name="uta")
            wv = (misc_pool.tile([128, 4 * C], bf16, tag="wvA", name="wvA"),
                  misc_pool.tile([128, 4 * C], bf16, tag="wvB", name="wvB"))
            for hh in range(2):
                nc.sync.dma_start(
                    out=wv[hh][:].rearrange("p (a m) -> p a m", a=4),
                    in_=wvT[512 * hh:512 * (hh + 1), :].rearrange(
                        "(a p) m -> p a m", p=128))
            QT = [qt_pool.tile([128, 1024], bf16, tag=f"qt{i}", name=f"qt{i}")
                  for i in range(8)]

            msk = misc_pool.tile([128, NMASK * 128], bf16, name="msk")
            nc.sync.dma_start(
                out=msk[:].rearrange("p (u m) -> p u m", u=NMASK),
                in_=masks[:, :, :].rearrange("u p m -> p u m"),
            )
            ones = misc_pool.tile([128, 2], bf16, name="ones")
            nc.vector.memset(ones[:], 1.0)

            def wslice(wh, ci, lo, hi):
                return wh[ci // 4][:, C * (ci % 4) + lo: C * (ci % 4) + hi]

            with tc.tile_pool(name="wp", bufs=2) as w_pool, \
                 tc.tile_pool(name="xp", bufs=1) as x_pool, \
                 tc.tile_pool(name="locp", bufs=1) as loc_pool, \
                 tc.tile_pool(name="dram", bufs=1, space="DRAM") as dram, \
                 tc.psum_pool(name="pproj", bufs=4) as pp:

                # ---- phase Q: QT[co] = wq.T @ xq -------------------------
                wq = load_w(w_pool, "wq", wqT)
                xq = load_xfull(x_pool, "xq", xqT)
                for i in range(2):
                    for co in range(8):
                        ps = pp.tile([128, 512], f32, tag="pk", name=f"qps{i}_{co}")
                        for ci in range(8):
                            nc.tensor.matmul(
                                ps[:],
                                wslice(wq, ci, 128 * co, 128 * (co + 1)),
                                xq[:, 1024 * ci + 512 * i:1024 * ci + 512 * (i + 1)],
                                start=(ci == 0), stop=(ci == 7),
                            )
                        nc.scalar.copy(QT[co][:, 512 * i:512 * (i + 1)], ps[:])

            # ---- attention ----------------------------------------------
            with tc.tile_pool(name="ptp", bufs=3) as pt_pool, \
                 tc.tile_pool(name="outp", bufs=2) as out_pool, \
                 tc.tile_pool(name="linvp", bufs=2) as linv_pool, \
                 tc.psum_pool(name="sp", bufs=3) as sp, \
                 tc.psum_pool(name="op", bufs=1) as op, \
                 tc.psum_pool(name="op2", bufs=1) as op2, \
                 tc.psum_pool(name="lp", bufs=1) as lp:

                def do_S_pair(s, t, n):
                    # S chains for units (2t, 2t+1) into one [128,256] psum;
                    # a single exp (and mask-add for the last pair) covers both
                    s_ps = sp.tile([128, 256], f32, tag="s", name=f"s{s}_{t}")
                    for u in range(2):
                        j = 2 * t + u
                        for ci in range(8):
                            nc.tensor.matmul(
                                s_ps[:, 128 * u:128 * (u + 1)],
                                xt_all[:, T * ci + 128 * j:T * ci + 128 * (j + 1)],
                                QT[ci][:, 128 * s:128 * (s + 1)],
                                start=(ci == 0), stop=(ci == 7),
                            )
                    if t == n // 2 - 1:
                        sm = pt_pool.tile([128, 256], f32, tag="sm",
                                          name=f"sm{s}_{t}")
                        nc.vector.tensor_add(sm[:], s_ps[:],
                                             msk[:, 256 * s:256 * (s + 1)])
                        src = sm
                    else:
                        src = s_ps
                    pm = pt_pool.tile([128, 256], bf16, tag="pm", name=f"pm{s}_{t}")
                    nc.scalar.activation(pm[:], src[:], Exp, scale=SCALE)
                    return pm

                for s in range(NSLOT):
                    n = SLOT_UNITS[s]
                    o_ps = op.tile([128, C], f32, tag="o", name=f"o{s}")
                    l_ps = lp.tile([128, 2], f32, tag="l", name=f"l{s}")

                    for t in range(n // 2):
                        pm = do_S_pair(s, t, n)
                        for u in range(2):
                            # U^T[c,q] += sum_k x[k,c] pm[k,q] (raw-x values)
                            j = 2 * t + u
                            pmu = pm[:, 128 * u:128 * (u + 1)]
                            first, last = (j == 0), (j == n - 1)
                            for cc in range(8):
                                # one start per psum zero region (bank):
                                # sibling chunks accumulate onto its zeroing
                                nc.tensor.matmul(
                                    o_ps[:, 128 * cc:128 * (cc + 1)],
                                    xk_all[:, C * j + 128 * cc:
                                           C * j + 128 * (cc + 1)],
                                    pmu, start=(first and cc % 4 == 0),
                                    stop=(last and cc % 4 == 3),
                                    skip_group_check=True)
                            nc.tensor.matmul(l_ps[:], pmu, ones[:],
                                             start=first, stop=last)

                    # drain U^T to SBUF for the folded Wv projection
                    for cc in range(8):
                        dst = ut_all[:, 1024 * cc + 128 * s:
                                     1024 * cc + 128 * (s + 1)]
                        if cc % 2 == 0:
                            nc.scalar.copy(dst, o_ps[:, 128 * cc:128 * (cc + 1)])
                        else:
                            nc.vector.tensor_copy(dst,
                                                  o_ps[:, 128 * cc:128 * (cc + 1)])

                    linv = linv_pool.tile([128, 1], f32, tag="linv",
                                          name=f"linv{s}")
                    nc.vector.reciprocal(linv[:], l_ps[:, 0:1])
                    o2 = op2.tile([128, C], f32, tag="o2", name=f"o2_{s}")
                    for half in range(2):
                        for cc in range(8):
                            nc.tensor.matmul(
                                o2[:, 512 * half:512 * (half + 1)],
                                ut_all[:, 1024 * cc + 128 * s:
                                       1024 * cc + 128 * (s + 1)],
                                wv[cc // 4][:, C * (cc % 4) + 512 * half:
                                            C * (cc % 4) + 512 * (half + 1)],
                                start=(cc == 0), stop=(cc == 7))
                    for half in range(2):
                        o_sb = out_pool.tile([128, 512], f32, tag=f"ost{half}",
                                             name=f"ost{s}_{half}")
                        nc.scalar.activation(o_sb[:],
                                             o2[:, 512 * half:512 * (half + 1)],
                                             Copy, scale=linv[:])
                        nc.sync.dma_start(
                            out=out[128 * s:128 * (s + 1),
                                    512 * half:512 * (half + 1)],
                            in_=o_sb[:],
                        )
    nc.finalize()
    return nc


def _masks_for_half(h):
    import ml_dtypes
    m = np.zeros((NMASK, 128, 128), np.float32)
    for s in range(NSLOT):
        n = SLOT_UNITS[s]
        g = ASSIGN[h][s]
        for d in range(2):
            j = n - 2 + d
            ks = 128 * j + np.arange(128)[:, None]
            qs = 128 * g + np.arange(128)[None, :]
            m[2 * s + d] = np.where(ks <= qs, 0.0, -30000.0)
    return m.astype(ml_dtypes.bfloat16)


def _get_built():
    if "nc" not in _CACHE:
        _CACHE["nc"] = _build_nc()
        _CACHE["masks"] = {h: _masks_for_half(h) for h in (0, 1)}
    return _CACHE["nc"], _CACHE["masks"]


def make_in_maps(x, Wk, Wq, Wv, mks):
    import ml_dtypes
    bf = ml_dtypes.bfloat16
    x = np.asarray(x, np.float32)
    m_fold = np.asarray(Wq, np.float32).T @ np.asarray(Wk, np.float32)
    wqT = np.ascontiguousarray(m_fold.astype(bf))
    wvT = np.ascontiguousarray(np.asarray(Wv, np.float32).T.astype(bf))

    in_maps = []
    for core in range(8):
        b, h = core // 2, core % 2
        xT_b = np.ascontiguousarray(x[b].T.astype(bf))
        gs = ASSIGN[h]
        xqT = np.ascontiguousarray(
            np.concatenate([xT_b[:, 128 * g:128 * (g + 1)] for g in gs], axis=1)
        )
        xN = np.ascontiguousarray(x[b].astype(bf))
        in_maps.append({
            "xT": xT_b, "xN": xN, "xqT": xqT,
            "wqT": wqT, "wvT": wvT,
            "masks": mks[h],
        })
    return in_maps


def kernel(x, Wk, Wq, Wv, **_ignored):
    from concourse.bass_utils import run_bass_kernel_spmd

    nc, mks = _get_built()
    in_maps = make_in_maps(x, Wk, Wq, Wv, mks)
    res = run_bass_kernel_spmd(nc, in_maps, core_ids=list(range(8)))
    _CACHE["last_res"] = res

    out = np.empty((B, T, C), np.float32)
    for core in range(8):
        b, h = core // 2, core % 2
        o = res.results[core]["out"]
        for s, g in enumerate(ASSIGN[h]):
            out[b, 128 * g:128 * (g + 1), :] = o[128 * s:128 * (s + 1), :]
    return out


# revision 7
# speedup vs baseline: 1.3764x; 1.0281x over previous
"""Single-head causal attention (B=4, T=2048, C=1024, fp32) on 8 Trainium2 cores.

v4 = v3 + pairwise K/V exchange: the two cores of a batch each compute K and
V for only their own half of the sequence (k rows [1024h, 1024h+1024)), then
swap halves with an AllGather over replica pairs [[0,1],[2,3],[4,5],[6,7]]
through DRAM bounce buffers. This halves the K and V projection matmul work
per core (the baseline computed full K,V on both cores of a pair).

Phase order V-half -> K-half -> Q so each exchange is in flight while the
next projection phase computes; gathered halves are read back rank-major,
which equals k-major, so attention addressing is core-uniform (SPMD).

Everything else (bf16 compute path, 8 slots of 128-row q-blocks with 16-2s
k-units, masks on last 2 units, psum-direct exp, no running max) is as v3.
"""

import os
import sys

import numpy as np

for _p in ("/opt/trn_rl_repo", os.path.expanduser("~/.axon_site/_ro/trn_rl_repo")):
    if os.path.isdir(_p) and _p not in sys.path:
        sys.path.insert(0, _p)

B, T, C = 4, 2048, 1024
NSLOT = 8
SLOT_UNITS = [16 - 2 * s for s in range(NSLOT)]      # [16,14,12,10,8,6,4,2]
ASSIGN = {
    0: [(15 - 2 * s) if s % 2 == 0 else (14 - 2 * s) for s in range(NSLOT)],
    1: [(14 - 2 * s) if s % 2 == 0 else (15 - 2 * s) for s in range(NSLOT)],
}
SCALE = float(C) ** -0.5
NMASK = 2 * NSLOT
PAIRS = [[0, 1], [2, 3], [4, 5], [6, 7]]

_CACHE = {}


def _build_nc(reps=1):
    import concourse.tile as tile
    from concourse import bacc, mybir
    from contextlib import ExitStack

    f32 = mybir.dt.float32
    bf16 = mybir.dt.bfloat16
    Exp = mybir.ActivationFunctionType.Exp
    Copy = mybir.ActivationFunctionType.Copy

    nc = bacc.Bacc("TRN2", target_bir_lowering=False, debug=False)

    # xhT: this core's own k-half of x^T; xqT: this core's q-blocks of x^T
    xT = nc.dram_tensor("xT", [C, T], bf16, kind="ExternalInput").ap()
    xN = nc.dram_tensor("xN", [T, C], bf16, kind="ExternalInput").ap()
    xqT = nc.dram_tensor("xqT", [C, 1024], bf16, kind="ExternalInput").ap()
    # "wqT" carries M = Wq^T @ Wk (host-folded): S = (x_q M) x^T
    wqT = nc.dram_tensor("wqT", [C, C], bf16, kind="ExternalInput").ap()
    wvT = nc.dram_tensor("wvT", [C, C], bf16, kind="ExternalInput").ap()
    masks = nc.dram_tensor("masks", [NMASK, 128, 128], bf16, kind="ExternalInput").ap()
    out = nc.dram_tensor("out", [1024, C], f32, kind="ExternalOutput").ap()

    def load_half(pool, tag, name, dram_ap, cols, bufs=None):
        t = pool.tile([128, 4 * cols], bf16, tag=tag, name=name, bufs=bufs)
        nc.sync.dma_start(
            out=t[:].rearrange("p (a m) -> p a m", a=4),
            in_=dram_ap.rearrange("(a p) m -> p a m", p=128),
        )
        return t

    def load_w(pool, name, dram_ap):
        return (load_half(pool, "wA", name + "A", dram_ap[0:512, :], C),
                load_half(pool, "wB", name + "B", dram_ap[512:1024, :], C))

    def load_xfull(pool, tag, dram_ap):
        """[1024, 1024] DRAM -> one [128, 8*1024] bf16 tile, two half DMAs."""
        t = pool.tile([128, 8 * 1024], bf16, tag=tag, name=tag)
        for hh in range(2):
            nc.sync.dma_start(
                out=t[:, 4096 * hh:4096 * (hh + 1)].rearrange(
                    "p (a m) -> p a m", a=4),
                in_=dram_ap[512 * hh:512 * (hh + 1), :].rearrange(
                    "(a p) m -> p a m", p=128),
            )
        return t

    with tile.TileContext(nc) as tc:
      for rep in range(reps):
        with ExitStack() as ctx:
            # ---- persistent SBUF arrays ---------------------------------
            kt_pool = ctx.enter_context(tc.tile_pool(name="ktp", bufs=1))
            v_pool = ctx.enter_context(tc.tile_pool(name="vp", bufs=1))
            qt_pool = ctx.enter_context(tc.tile_pool(name="qtp", bufs=1))
            misc_pool = ctx.enter_context(tc.tile_pool(name="miscp", bufs=1))

            xt_all = kt_pool.tile([128, 8 * T], bf16, tag="xta", name="xta")
            xk_all = v_pool.tile([128, 16 * C], bf16, tag="xka", name="xka")
            ut_all = v_pool.tile([128, 8 * 1024], bf16, tag="uta", name="uta")
            wv = (misc_pool.tile([128, 4 * C], bf16, tag="wvA", name="wvA"),
                  misc_pool.tile([128, 4 * C], bf16, tag="wvB", name="wvB"))

            QT = [qt_pool.tile([128, 1024], bf16, tag=f"qt{i}", name=f"qt{i}")
                  for i in range(8)]

            msk = misc_pool.tile([128, NMASK * 128], bf16, name="msk")
            ones = misc_pool.tile([128, 2], bf16, name="ones")
            nc.vector.memset(ones[:], 1.0)

            def wslice(wh, ci, lo, hi):
                return wh[ci // 4][:, C * (ci % 4) + lo: C * (ci % 4) + hi]

            with tc.tile_pool(name="wp", bufs=2) as w_pool, \
                 tc.tile_pool(name="xp", bufs=1) as x_pool, \
                 tc.tile_pool(name="locp", bufs=1) as loc_pool, \
                 tc.tile_pool(name="dram", bufs=1, space="DRAM") as dram, \
                 tc.psum_pool(name="pproj", bufs=4) as pp:

                # ---- phase Q: QT[co] = wq.T @ xq -------------------------
                wq = load_w(w_pool, "wq", wqT)
                xq = load_xfull(x_pool, "xq", xqT)
                for i in range(2):
                    for co in range(8):
                        ps = pp.tile([128, 512], f32, tag="pk", name=f"qps{i}_{co}")
                        for ci in range(8):
                            nc.tensor.matmul(
                                ps[:],
                                wslice(wq, ci, 128 * co, 128 * (co + 1)),
                                xq[:, 1024 * ci + 512 * i:1024 * ci + 512 * (i + 1)],
                                start=(ci == 0), stop=(ci == 7),
                            )
                        nc.scalar.copy(QT[co][:, 512 * i:512 * (i + 1)], ps[:])

            # attention-phase data loads, emitted after the XM loads so the
            # XM phase starts ~25us earlier (need order: xt, xk, masks, wv)
            for hh in range(2):
                nc.sync.dma_start(
                    out=xt_all[:].rearrange("p (a m) -> p a m", a=8)[
                        :, 4 * hh:4 * (hh + 1), :],
                    in_=xT[512 * hh:512 * (hh + 1), :].rearrange(
                        "(a p) m -> p a m", p=128))
            nc.sync.dma_start(
                out=xk_all[:].rearrange("p (a m) -> p a m", a=16),
                in_=xN.rearrange("(a p) m -> p a m", p=128))
            nc.sync.dma_start(
                out=msk[:].rearrange("p (u m) -> p u m", u=NMASK),
                in_=masks[:, :, :].rearrange("u p m -> p u m"),
            )
            for hh in range(2):
                nc.sync.dma_start(
                    out=wv[hh][:].rearrange("p (a m) -> p a m", a=4),
                    in_=wvT[512 * hh:512 * (hh + 1), :].rearrange(
                        "(a p) m -> p a m", p=128))

            # ---- attention ----------------------------------------------
            with tc.tile_pool(name="ptp", bufs=3) as pt_pool, \
                 tc.tile_pool(name="outp", bufs=2) as out_pool, \
                 tc.tile_pool(name="linvp", bufs=2) as linv_pool, \
                 tc.psum_pool(name="sp", bufs=3) as sp, \
                 tc.psum_pool(name="op", bufs=1) as op, \
                 tc.psum_pool(name="op2", bufs=1) as op2, \
                 tc.psum_pool(name="lp", bufs=1) as lp:

                def do_S_pair(s, t, n):
                    # S chains for units (2t, 2t+1) into one [128,256] psum;
                    # a single exp (and mask-add for the last pair) covers both
                    s_ps = sp.tile([128, 256], f32, tag="s", name=f"s{s}_{t}")
                    for u in range(2):
                        j = 2 * t + u
                        for ci in range(8):
                            nc.tensor.matmul(
                                s_ps[:, 128 * u:128 * (u + 1)],
                                xt_all[:, T * ci + 128 * j:T * ci + 128 * (j + 1)],
                                QT[ci][:, 128 * s:128 * (s + 1)],
                                start=(ci == 0), stop=(ci == 7),
                            )
                    if t == n // 2 - 1:
                        sm = pt_pool.tile([128, 256], f32, tag="sm",
                                          name=f"sm{s}_{t}")
                        nc.vector.tensor_add(sm[:], s_ps[:],
                                             msk[:, 256 * s:256 * (s + 1)])
                        src = sm
                    else:
                        src = s_ps
                    pm = pt_pool.tile([128, 256], bf16, tag="pm", name=f"pm{s}_{t}")
                    nc.scalar.activation(pm[:], src[:], Exp, scale=SCALE)
                    return pm

                for s in range(NSLOT):
                    n = SLOT_UNITS[s]
                    o_ps = op.tile([128, C], f32, tag="o", name=f"o{s}")
                    l_ps = lp.tile([128, 2], f32, tag="l", name=f"l{s}")

                    for t in range(n // 2):
                        pm = do_S_pair(s, t, n)
                        for u in range(2):
                            # U^T[c,q] += sum_k x[k,c] pm[k,q] (raw-x values)
                            j = 2 * t + u
                            pmu = pm[:, 128 * u:128 * (u + 1)]
                            first, last = (j == 0), (j == n - 1)
                            for cc in range(8):
                                # one start per psum zero region (bank):
                                # sibling chunks accumulate onto its zeroing
                                nc.tensor.matmul(
                                    o_ps[:, 128 * cc:128 * (cc + 1)],
                                    xk_all[:, C * j + 128 * cc:
                                           C * j + 128 * (cc + 1)],
                                    pmu, start=(first and cc % 4 == 0),
                                    stop=(last and cc % 4 == 3),
                                    skip_group_check=True)
                            nc.tensor.matmul(l_ps[:], pmu, ones[:],
                                             start=first, stop=last)

                    # drain U^T to SBUF for the folded Wv projection
                    for cc in range(8):
                        dst = ut_all[:, 1024 * cc + 128 * s:
                                     1024 * cc + 128 * (s + 1)]
                        if cc % 2 == 0:
                            nc.scalar.copy(dst, o_ps[:, 128 * cc:128 * (cc + 1)])
                        else:
                            nc.vector.tensor_copy(dst,
                                                  o_ps[:, 128 * cc:128 * (cc + 1)])

                    linv = linv_pool.tile([128, 1], f32, tag="linv",
                                          name=f"linv{s}")
                    nc.vector.reciprocal(linv[:], l_ps[:, 0:1])
                    o2 = op2.tile([128, C], f32, tag="o2", name=f"o2_{s}")
                    for half in range(2):
                        for cc in range(8):
                            nc.tensor.matmul(
                                o2[:, 512 * half:512 * (half + 1)],
                                ut_all[:, 1024 * cc + 128 * s:
                                       1024 * cc + 128 * (s + 1)],
                                wv[cc // 4][:, C * (cc % 4) + 512 * half:
                                            C * (cc % 4) + 512 * (half + 1)],
                                start=(cc == 0), stop=(cc == 7))
                    for half in range(2):
                        o_sb = out_pool.tile([128, 512], f32, tag=f"ost{half}",
                                             name=f"ost{s}_{half}")
                        nc.scalar.activation(o_sb[:],
                                             o2[:, 512 * half:512 * (half + 1)],
                                             Copy, scale=linv[:])
                        nc.sync.dma_start(
                            out=out[128 * s:128 * (s + 1),
                                    512 * half:512 * (half + 1)],
                            in_=o_sb[:],
                        )
    nc.finalize()
    return nc


def _masks_for_half(h):
    import ml_dtypes
    m = np.zeros((NMASK, 128, 128), np.float32)
    for s in range(NSLOT):
        n = SLOT_UNITS[s]
        g = ASSIGN[h][s]
        for d in range(2):
            j = n - 2 + d
            ks = 128 * j + np.arange(128)[:, None]
            qs = 128 * g + np.arange(128)[None, :]
            m[2 * s + d] = np.where(ks <= qs, 0.0, -30000.0)
    return m.astype(ml_dtypes.bfloat16)


def _get_built():
    if "nc" not in _CACHE:
        _CACHE["nc"] = _build_nc()
        _CACHE["masks"] = {h: _masks_for_half(h) for h in (0, 1)}
    return _CACHE["nc"], _CACHE["masks"]


def make_in_maps(x, Wk, Wq, Wv, mks):
    import ml_dtypes
    bf = ml_dtypes.bfloat16
    x = np.asarray(x, np.float32)
    m_fold = np.asarray(Wq, np.float32).T @ np.asarray(Wk, np.float32)
    wqT = np.ascontiguousarray(m_fold.astype(bf))
    wvT = np.ascontiguousarray(np.asarray(Wv, np.float32).T.astype(bf))

    in_maps = []
    for core in range(8):
        b, h = core // 2, core % 2
        xT_b = np.ascontiguousarray(x[b].T.astype(bf))
        gs = ASSIGN[h]
        xqT = np.ascontiguousarray(
            np.concatenate([xT_b[:, 128 * g:128 * (g + 1)] for g in gs], axis=1)
        )
        xN = np.ascontiguousarray(x[b].astype(bf))
        in_maps.append({
            "xT": xT_b, "xN": xN, "xqT": xqT,
            "wqT": wqT, "wvT": wvT,
            "masks": mks[h],
        })
    return in_maps


def kernel(x, Wk, Wq, Wv, **_ignored):
    from concourse.bass_utils import run_bass_kernel_spmd

    nc, mks = _get_built()
    in_maps = make_in_maps(x, Wk, Wq, Wv, mks)
    res = run_bass_kernel_spmd(nc, in_maps, core_ids=list(range(8)))
    _CACHE["last_res"] = res

    out = np.empty((B, T, C), np.float32)
    for core in range(8):
        b, h = core // 2, core % 2
        o = res.results[core]["out"]
        for s, g in enumerate(ASSIGN[h]):
            out[b, 128 * g:128 * (g + 1), :] = o[128 * s:128 * (s + 1), :]
    return out
